# revision 15
# baseline (speedup 1.0000x reference)
"""ChebConv K=2 (L_hat = -D^-1/2 A D^-1/2) distributed over 8 NeuronCores.

Sharding: nodes 12500/core; edges partitioned by destination shard.

Key identity: with coef_e = -w_e * dinv[row_e],
  Tx1 = segment_sum(norm * x[row], col) = (S^T Xg) @ W1,  scaled by dinv[col],
where Xg[slot] = coef_e * x[row_e] is the slot-ordered, coefficient-scaled
resharding of the INPUT x (host-side input sharding), and S is the one-hot
slot->dest scatter matrix, built ON-CHIP per 128x128 instance via a DVE
iota-compare. W1 commutes out of the scatter, so no per-edge z gather is
needed at all -- the device never runs a data-dependent DMA.

Launch structure:
  L1 (row-sharded edges): deg = segment_sum(w, row) via a padded per-node
     weight table + one batched free-dim reduce; dinv = deg>0 ? rsqrt(deg) : 0.
  host: coef_e = -w_e * dinv[row_e]; builds per-core slot tables
     xgs[k, tile, f] (fp16, PE lhsT layout) plus slotenc (x2-duplicated for
     DVE 2x mode) and a dest-side dinvT [64, NSH] replica.
  L2 (dest-sharded edges): per dest group g accumulates
     Yt_g = sum_insts xgs_tile^T @ mask_inst  ([64 feat, 128 dst] in PSUM),
     yts_g = Yt_g * dinvT_g (DVE, fp16), then
     outT_g = W1^T @ yts_g + W0a^T @ xta_g  (PSUM accumulate, includes bias),
     written transposed [64, NSH]; host transposes back.

The edge schedule is equalized across cores (per-group slot counts = max over
cores, padded to whole 128-slot tiles so instance == tile) so one SPMD kernel
serves all 8 cores; shortfall slots have zero xgs rows and sentinel slotenc,
contributing 0.
"""
import sys

if "/opt/trn_rl_repo" not in sys.path:
    sys.path.insert(0, "/opt/trn_rl_repo")

import numpy as np

import concourse.bass as bass
import concourse.bacc as bacc
import concourse.mybir as mybir
import concourse.tile as tile
from concourse.bass_utils import run_bass_kernel_spmd

P = 128
D = 64
N_NODES = 100000
N_CORES = 8
NSH = N_NODES // N_CORES            # 12500 nodes per shard
NG = (NSH + P - 1) // P             # 98 dest groups per shard
SG_GROUPS = 8                       # groups per super-group
NSG = (NG + SG_GROUPS - 1) // SG_GROUPS
NB = 2                              # buckets (kept for schedule shape only)
BSPLIT = 65536
SENTINEL = 999.0                    # slotenc for "not a member of this inst"

F32 = mybir.dt.float32
F16 = mybir.dt.float16

_cache = {}
LAST_STATS = {}


# ----------------------------------------------------------------- L1 kernel
def build_l1(kd):
    nc = bacc.Bacc("TRN2", target_bir_lowering=False, debug=False,
                   num_devices=N_CORES)
    wpad_d = nc.dram_tensor("wpad", [P, NG * kd], F16, kind="ExternalInput")
    dinv_d = nc.dram_tensor("dinv", [P, NG], F32, kind="ExternalOutput")

    with tile.TileContext(nc) as tc:
        with tc.tile_pool(name="const", bufs=1) as cpool:
            wbig = cpool.tile([P, NG, kd], F16)
            nc.sync.dma_start(wbig[:], wpad_d[:, :])
            deg_t = cpool.tile([P, NG], F32)
            nc.vector.reduce_sum(deg_t[:], wbig[:], axis=mybir.AxisListType.X)
            m_t = cpool.tile([P, NG], F32)
            nc.vector.tensor_scalar_max(m_t[:], deg_t[:], 1e-30)
            s_t = cpool.tile([P, NG], F32)
            nc.scalar.activation(s_t[:], m_t[:], mybir.ActivationFunctionType.Sqrt)
            r_t = cpool.tile([P, NG], F32)
            nc.vector.reciprocal(r_t[:], s_t[:])
            mask_t = cpool.tile([P, NG], F32)
            nc.vector.tensor_scalar(
                out=mask_t[:], in0=deg_t[:], scalar1=0.0, scalar2=None,
                op0=mybir.AluOpType.is_gt,
            )
            dinv_t = cpool.tile([P, NG], F32)
            nc.vector.tensor_mul(dinv_t[:], r_t[:], mask_t[:])
            nc.sync.dma_start(dinv_d[:, :], dinv_t[:])
    nc.compile()
    return nc


# ----------------------------------------------------------------- L2 kernel
def build_l2(sched):
    (calls, ginsts, sg_tiles, tile_base, sg_insts, grp_insts,
     tot16, tot_inst, tot_tiles) = sched

    nc = bacc.Bacc("TRN2", target_bir_lowering=False, debug=False,
                   num_devices=N_CORES)
    xgs_d = nc.dram_tensor("xgs", [P, tot_tiles * D], F16,
                           kind="ExternalInput")
    xta_d = nc.dram_tensor("xta", [D + 1, NSH], F16, kind="ExternalInput")
    w0a_d = nc.dram_tensor("w0a", [D + 1, D], F16, kind="ExternalInput")
    w1_d = nc.dram_tensor("w1", [D, D], F16, kind="ExternalInput")
    dvt_d = nc.dram_tensor("dinvT", [D, NSH], F16, kind="ExternalInput")
    se_d = nc.dram_tensor("slotenc", [P, tot_inst * 2], F16,
                          kind="ExternalInput")
    io_d = nc.dram_tensor("iota", [P, P], F16, kind="ExternalInput")
    out_d = nc.dram_tensor("outT", [D, NSH], F32, kind="ExternalOutput")

    with tile.TileContext(nc) as tc:
        with (
            tc.tile_pool(name="const", bufs=1) as cpool,
            tc.tile_pool(name="xg", bufs=2) as xpool,
            tc.tile_pool(name="mk", bufs=2) as mpool,
            tc.tile_pool(name="io", bufs=2) as iopool,
            tc.tile_pool(name="yt", bufs=2) as ypool,
            tc.tile_pool(name="psy", bufs=1, space="PSUM") as psy_pool,
            tc.tile_pool(name="pso", bufs=2, space="PSUM") as pso_pool,
        ):
            io_t = cpool.tile([P, P], F16)
            nc.sync.dma_start(io_t[:], io_d[:, :])
            w0a_t = cpool.tile([D + 1, D], F16)
            nc.sync.dma_start(w0a_t[:], w0a_d[:, :])
            w1_t = cpool.tile([D, D], F16)
            nc.sync.dma_start(w1_t[:], w1_d[:, :])
            dvt_t = cpool.tile([D, NSH], F16)
            nc.sync.dma_start(dvt_t[:], dvt_d[:, :])
            se_t = cpool.tile([P, tot_inst * 2], F16)
            nc.sync.dma_start(se_t[:], se_d[:, :])

            pending = None  # (po, g0, g1)

            def flush_pending():
                nonlocal pending
                if pending is None:
                    return
                po, g0, g1 = pending
                ng = g1 - g0
                n0 = g0 * P
                n1 = min(g1 * P, NSH)
                ot = iopool.tile([D, SG_GROUPS * P], F32, tag="o")
                nc.scalar.activation(ot[:, :ng * P], po[:, :ng, :],
                                     mybir.ActivationFunctionType.Copy)
                nc.sync.dma_start(out_d[:, n0:n1], ot[:, :n1 - n0])
                pending = None

            slabs = {}

            def emit_slab(sg):
                # one is_equal over the whole supergroup's instances:
                # amortizes the per-op DVE fixed cost 8x
                ilo, ihi = sg_insts[sg]
                ninst = ihi - ilo
                m_t = mpool.tile([P, ninst, P], F16, tag="m")
                sa = se_t[:, ilo * 2:ihi * 2]
                in0 = bass.AP(sa.tensor, sa.offset,
                              [sa.ap[0], [2, ninst], [0, D], [1, 2]])
                ia = io_t[:, :]
                in1 = bass.AP(ia.tensor, ia.offset,
                              [ia.ap[0], [0, ninst], [2, D], [1, 2]])
                ma = m_t[:, :, :]
                mo = bass.AP(ma.tensor, ma.offset,
                             [ma.ap[0], [P, ninst], [2, D], [1, 2]])
                nc.vector.tensor_tensor(out=mo, in0=in0, in1=in1,
                                        op=mybir.AluOpType.is_equal)
                slabs[sg] = m_t

            emit_slab(0)
            for sg in range(NSG):
                g0 = sg * SG_GROUPS
                g1 = min(g0 + SG_GROUPS, NG)
                ng = g1 - g0
                ilo, ihi = sg_insts[sg]
                tb = tile_base[sg]
                ntiles = sg_tiles[sg]

                xg_t = xpool.tile([P, ntiles, D], F16, tag="x")
                nc.sync.dma_start(xg_t[:], xgs_d[:, tb * D:(tb + ntiles) * D])
                xta_t = iopool.tile([D + 1, SG_GROUPS * P], F16, tag="xa")
                nsg_n = min(g1 * P, NSH) - g0 * P
                nc.sync.dma_start(xta_t[:, :nsg_n],
                                  xta_d[:, g0 * P:g0 * P + nsg_n])

                m_t = slabs.pop(sg)
                if sg + 1 < NSG:
                    emit_slab(sg + 1)    # DVE: runs during this sg's PE work
                flush_pending()

                po = pso_pool.tile([D, SG_GROUPS, P], F32, tag="o",
                                   space="PSUM")
                for g in range(g0, g1):
                    gg = g - g0
                    n0 = g * P
                    np_ = min(n0 + P, NSH) - n0
                    py = psy_pool.tile([D, P], F32, tag=f"y{g % 3}",
                                       space="PSUM")
                    insts = ginsts[g]
                    kb = len(insts)
                    for j, (inst_id, t) in enumerate(insts):
                        nc.tensor.matmul(
                            out=py[:, :],
                            lhsT=xg_t[:, t, :],
                            rhs=m_t[:, inst_id - ilo, :],
                            start=(j == 0),
                            stop=(j == kb - 1),
                        )
                    y_t = ypool.tile([D, P], F16, tag=f"t{g % 3}")
                    nc.vector.tensor_tensor(
                        out=y_t[:, :np_], in0=py[:, :np_],
                        in1=dvt_t[:, n0:n0 + np_],
                        op=mybir.AluOpType.mult)
                    nc.tensor.matmul(out=po[:, gg, :np_], lhsT=w1_t[:],
                                     rhs=y_t[:, :np_],
                                     start=True, stop=False)
                    nc.tensor.matmul(out=po[:, gg, :np_], lhsT=w0a_t[:],
                                     rhs=xta_t[:, gg * P:gg * P + np_],
                                     start=False, stop=True)
                pending = (po, g0, g1)
            flush_pending()
    nc.compile()
    return nc


# ------------------------------------------------------------- host prep
def _prep_l1(row, w):
    """Per-core padded weight tables. Returns (kd, list of [P, NG*kd] f16)."""
    core = row // NSH
    data = []
    kd = 4
    for c in range(N_CORES):
        sel = core == c
        r_loc = (row[sel] - c * NSH).astype(np.int64)
        w_c = w[sel]
        counts = np.bincount(r_loc, minlength=NSH)
        kd = max(kd, int(counts.max()))
        data.append((r_loc, w_c, counts))
    kd = ((kd + 3) // 4) * 4
    out = []
    for r_loc, w_c, counts in data:
        offs = np.cumsum(counts) - counts
        order = np.argsort(r_loc, kind="stable")
        r_s = r_loc[order]
        w_s = w_c[order]
        k = np.arange(len(r_s)) - offs[r_s]
        wpad = np.zeros((NG * P, kd), np.float16)
        wpad[r_s, k] = w_s
        wbig = wpad.reshape(NG, P, kd).transpose(1, 0, 2).reshape(P, NG * kd)
        out.append(np.ascontiguousarray(wbig))
    return kd, out


def _prep_l2(row, col, w):
    """Core-equalized, group-padded L2 schedule (no buckets, no straddles:
    each group's slots are padded to full 128-tiles, so instance == tile).

    Returns (sched, percore) where percore[c] holds slot assignments.
    """
    core = col // NSH
    percore = []
    counts = np.zeros((N_CORES, NG), np.int64)
    for c in range(N_CORES):
        sel = core == c
        loc = col[sel] - c * NSH
        percore.append((loc // P, loc % P, row[sel], w[sel]))
        counts[c] = np.bincount(loc // P, minlength=NG)
    smax = counts.max(axis=0)            # [NG] equalized group sizes
    gtiles = -(-smax // P)               # tiles per group

    seg_slot = np.zeros(NG, np.int64)    # global slot of group start
    sg_tiles = []
    tile_base = []
    ginsts = []
    grp_insts = []
    sg_insts = []
    gslot = 0
    inst_id = 0
    for sg in range(NSG):
        g0, g1 = sg * SG_GROUPS, min((sg + 1) * SG_GROUPS, NG)
        tile_base.append(gslot // P)
        sg_slot0 = gslot
        lo_sg = inst_id
        for g in range(g0, g1):
            seg_slot[g] = gslot
            nt = int(gtiles[g])
            t0 = (gslot - sg_slot0) // P
            lo = inst_id
            inst_id += nt
            grp_insts.append((lo, inst_id))
            ginsts.append(tuple((lo + j, t0 + j) for j in range(nt)))
            gslot += nt * P
        sg_tiles.append((gslot - sg_slot0) // P)
        sg_insts.append((lo_sg, inst_id))
    tot_slots = gslot
    tot_tiles = tot_slots // P
    tot_inst = inst_id

    sched = (None, tuple(ginsts), tuple(sg_tiles), tuple(tile_base),
             tuple(sg_insts), tuple(grp_insts), 0, tot_inst, tot_tiles)

    # --- per-core slot assignment ---
    slots = []
    for c in range(N_CORES):
        g_e, d_e, r_e, w_e = percore[c]
        cnt = counts[c]
        offs = np.cumsum(cnt) - cnt
        order = np.argsort(g_e, kind="stable")
        inv = np.empty_like(order)
        inv[order] = np.arange(len(order))
        pos_in_seg = inv - offs[g_e]
        slot_e = seg_slot[g_e] + pos_in_seg

        k_e = slot_e % P
        gtile_e = slot_e // P
        # instance id == global tile id by construction
        inst_e = gtile_e

        slotenc = np.full((P, tot_inst), SENTINEL, np.float16)
        slotenc[k_e, inst_e] = d_e.astype(np.float16)
        slots.append({"slot": slot_e, "rows": r_e, "w": w_e,
                      "slotenc": np.repeat(slotenc, 2, axis=1)})
    return sched, slots


# ------------------------------------------------------------------ kernel()
def kernel(x, edge_index, edge_weight, W0, W1, b):
    global LAST_STATS
    x = np.asarray(x, np.float32)
    edge_index = np.asarray(edge_index)
    w = np.asarray(edge_weight, np.float32)
    W0 = np.asarray(W0, np.float32)
    W1 = np.asarray(W1, np.float32)
    b = np.asarray(b, np.float32)
    row = edge_index[0].astype(np.int64)
    col = edge_index[1].astype(np.int64)

    kd, wpads = _prep_l1(row, w)
    sched, slots = _prep_l2(row, col, w)
    tot_tiles = sched[8]
    sched_key = (sched[0], sched[2], sched[4], sched[6], sched[7], sched[8])

    if ("l1", kd) not in _cache:
        _cache[("l1", kd)] = build_l1(kd)
    nc1 = _cache[("l1", kd)]
    if ("l2", sched_key) not in _cache:
        _cache[("l2", sched_key)] = build_l2(sched)
    nc2 = _cache[("l2", sched_key)]

    in1 = [{"wpad": wpads[c]} for c in range(N_CORES)]
    res1 = run_bass_kernel_spmd(nc1, in1, core_ids=list(range(N_CORES)))

    # dinv per node (host assembly of the L1 output)
    dinv_full = np.empty(N_NODES, np.float32)
    for c in range(N_CORES):
        dv = res1.results[c]["dinv"]          # [P, NG], node = g*128 + p
        dinv_full[c * NSH:(c + 1) * NSH] = dv.T.reshape(-1)[:NSH]

    w1h = W1.astype(np.float16)
    w0a = np.concatenate([W0, b.reshape(1, D)], axis=0).astype(np.float16)
    iota = np.tile(np.arange(P, dtype=np.float16), (P, 1))

    in2 = []
    for c in range(N_CORES):
        s = slots[c]
        coef = (-s["w"] * dinv_full[s["rows"]]).astype(np.float32)
        xg = np.zeros((tot_tiles * P, D), np.float16)
        xg[s["slot"]] = (x[s["rows"]] * coef[:, None]).astype(np.float16)
        xgs = np.ascontiguousarray(
            xg.reshape(tot_tiles, P, D).transpose(1, 0, 2)).reshape(P, -1)

        xs = x[c * NSH:(c + 1) * NSH]
        xta = np.ascontiguousarray(np.concatenate(
            [xs.T, np.ones((1, NSH), np.float32)], axis=0).astype(np.float16))
        dvt = np.ascontiguousarray(np.tile(
            dinv_full[c * NSH:(c + 1) * NSH].astype(np.float16), (D, 1)))
        in2.append({"xgs": xgs, "xta": xta, "w0a": w0a, "w1": w1h,
                    "dinvT": dvt, "slotenc": s["slotenc"], "iota": iota})
    res2 = run_bass_kernel_spmd(nc2, in2, core_ids=list(range(N_CORES)))
    out = np.concatenate(
        [res2.results[c]["outT"].T for c in range(N_CORES)], axis=0)
    LAST_STATS = {
        "l1_exec_ns": res1.exec_time_ns,
        "l2_exec_ns": res2.exec_time_ns,
        "insts": sched[7],
        "tiles": sched[8],
    }
    return np.ascontiguousarray(out.astype(np.float32))


# revision 19
# speedup vs baseline: 1.4722x; 1.4722x over previous
"""ChebConv K=2 (L_hat = -D^-1/2 A D^-1/2) distributed over 8 NeuronCores.

Sharding: nodes 12500/core; edges partitioned by destination shard.

Key identity: with coef_e = -w_e * dinv[row_e],
  Tx1 = segment_sum(norm * x[row], col) = (S^T Xg) @ W1,  scaled by dinv[col],
where Xg[slot] = coef_e * x[row_e] is the slot-ordered, coefficient-scaled
resharding of the INPUT x (host-side input sharding), and S is the one-hot
slot->dest scatter matrix, built ON-CHIP per 128x128 instance via a DVE
iota-compare. W1 commutes out of the scatter, so no per-edge z gather is
needed at all -- the device never runs a data-dependent DMA.

Launch structure:
  L1 (row-sharded edges): deg = segment_sum(w, row) via a padded per-node
     weight table + one batched free-dim reduce; dinv = deg>0 ? rsqrt(deg) : 0.
  host: coef_e = -w_e * dinv[row_e]; builds per-core slot tables
     xgs[k, tile, f] (fp16, PE lhsT layout) plus slotenc (x2-duplicated for
     DVE 2x mode) and a dest-side dinvT [64, NSH] replica.
  L2 (dest-sharded edges): per dest group g accumulates
     Yt_g = sum_insts xgs_tile^T @ mask_inst  ([64 feat, 128 dst] in PSUM),
     yts_g = Yt_g * dinvT_g (DVE, fp16), then
     outT_g = W1^T @ yts_g + W0a^T @ xta_g  (PSUM accumulate, includes bias),
     written transposed [64, NSH]; host transposes back.

The edge schedule is equalized across cores (per-group slot counts = max over
cores, padded to whole 128-slot tiles so instance == tile) so one SPMD kernel
serves all 8 cores; shortfall slots have zero xgs rows and sentinel slotenc,
contributing 0.
"""
import sys

if "/opt/trn_rl_repo" not in sys.path:
    sys.path.insert(0, "/opt/trn_rl_repo")

import numpy as np

import concourse.bass as bass
import concourse.bacc as bacc
import concourse.mybir as mybir
import concourse.tile as tile
from concourse.bass_utils import run_bass_kernel_spmd

P = 128
D = 64
N_NODES = 100000
N_CORES = 8
NSH = N_NODES // N_CORES            # 12500 nodes per shard
NG = (NSH + P - 1) // P             # 98 dest groups per shard
SG_GROUPS = 8                       # groups per super-group
NSG = (NG + SG_GROUPS - 1) // SG_GROUPS
NB = 2                              # buckets (kept for schedule shape only)
BSPLIT = 65536
SENTINEL = 999.0                    # slotenc for "not a member of this inst"

F32 = mybir.dt.float32
F16 = mybir.dt.float16

_cache = {}
LAST_STATS = {}


# ----------------------------------------------------------------- L1 kernel
def build_l1(kd):
    nc = bacc.Bacc("TRN2", target_bir_lowering=False, debug=False,
                   num_devices=N_CORES)
    wpad_d = nc.dram_tensor("wpad", [P, NG * kd], F16, kind="ExternalInput")
    dinv_d = nc.dram_tensor("dinv", [P, NG], F32, kind="ExternalOutput")

    with tile.TileContext(nc) as tc:
        with tc.tile_pool(name="const", bufs=1) as cpool:
            wbig = cpool.tile([P, NG, kd], F16)
            nc.sync.dma_start(wbig[:], wpad_d[:, :])
            deg_t = cpool.tile([P, NG], F32)
            nc.vector.reduce_sum(deg_t[:], wbig[:], axis=mybir.AxisListType.X)
            m_t = cpool.tile([P, NG], F32)
            nc.vector.tensor_scalar_max(m_t[:], deg_t[:], 1e-30)
            s_t = cpool.tile([P, NG], F32)
            nc.scalar.activation(s_t[:], m_t[:], mybir.ActivationFunctionType.Sqrt)
            r_t = cpool.tile([P, NG], F32)
            nc.vector.reciprocal(r_t[:], s_t[:])
            mask_t = cpool.tile([P, NG], F32)
            nc.vector.tensor_scalar(
                out=mask_t[:], in0=deg_t[:], scalar1=0.0, scalar2=None,
                op0=mybir.AluOpType.is_gt,
            )
            dinv_t = cpool.tile([P, NG], F32)
            nc.vector.tensor_mul(dinv_t[:], r_t[:], mask_t[:])
            nc.sync.dma_start(dinv_d[:, :], dinv_t[:])
    nc.compile()
    return nc


# ----------------------------------------------------------------- L2 kernel
def build_l2(sched):
    (calls, ginsts, sg_tiles, tile_base, sg_insts, grp_insts,
     tot16, tot_inst, tot_tiles) = sched

    nc = bacc.Bacc("TRN2", target_bir_lowering=False, debug=False,
                   num_devices=N_CORES)
    xgs_d = nc.dram_tensor("xgs", [P, tot_tiles * D], F16,
                           kind="ExternalInput")
    xta_d = nc.dram_tensor("xta", [D + 1, NSH], F16, kind="ExternalInput")
    w0a_d = nc.dram_tensor("w0a", [D + 1, D], F16, kind="ExternalInput")
    w1_d = nc.dram_tensor("w1", [D, D], F16, kind="ExternalInput")
    dvt_d = nc.dram_tensor("dinvT", [D, NSH], F16, kind="ExternalInput")
    se_d = nc.dram_tensor("slotenc", [P, tot_inst * 2], F16,
                          kind="ExternalInput")
    io_d = nc.dram_tensor("iota", [P, P], F16, kind="ExternalInput")
    out_d = nc.dram_tensor("outT", [D, NSH], F32, kind="ExternalOutput")

    with tile.TileContext(nc) as tc:
        with (
            tc.tile_pool(name="const", bufs=1) as cpool,
            tc.tile_pool(name="xg", bufs=3) as xpool,
            tc.tile_pool(name="mk", bufs=2) as mpool,
            tc.tile_pool(name="io", bufs=2) as iopool,
            tc.tile_pool(name="yt", bufs=2) as ypool,
            tc.tile_pool(name="psy", bufs=1, space="PSUM") as psy_pool,
            tc.tile_pool(name="pso", bufs=2, space="PSUM") as pso_pool,
        ):
            io_t = cpool.tile([P, P], F16)
            nc.sync.dma_start(io_t[:], io_d[:, :])
            w0a_t = cpool.tile([D + 1, D], F16)
            nc.sync.dma_start(w0a_t[:], w0a_d[:, :])
            w1_t = cpool.tile([D, D], F16)
            nc.sync.dma_start(w1_t[:], w1_d[:, :])
            dvt_t = cpool.tile([D, NSH], F16)
            nc.sync.dma_start(dvt_t[:], dvt_d[:, :])
            se_t = cpool.tile([P, tot_inst * 2], F16)
            nc.sync.dma_start(se_t[:], se_d[:, :])

            pending = None  # (po, g0, g1)

            def flush_pending():
                nonlocal pending
                if pending is None:
                    return
                po, g0, g1 = pending
                ng = g1 - g0
                n0 = g0 * P
                n1 = min(g1 * P, NSH)
                ot = iopool.tile([D, SG_GROUPS * P], F32, tag="o")
                nc.scalar.activation(ot[:, :ng * P], po[:, :ng, :],
                                     mybir.ActivationFunctionType.Copy)
                nc.sync.dma_start(out_d[:, n0:n1], ot[:, :n1 - n0])
                pending = None

            for sg in range(NSG):
                g0 = sg * SG_GROUPS
                g1 = min(g0 + SG_GROUPS, NG)
                ng = g1 - g0
                ilo, ihi = sg_insts[sg]
                tb = tile_base[sg]
                ntiles = sg_tiles[sg]

                xg_t = xpool.tile([P, ntiles, D], F16, tag="x")
                nc.sync.dma_start(xg_t[:], xgs_d[:, tb * D:(tb + ntiles) * D])
                xta_t = iopool.tile([D + 1, SG_GROUPS * P], F16, tag="xa")
                nsg_n = min(g1 * P, NSH) - g0 * P
                nc.sync.dma_start(xta_t[:, :nsg_n],
                                  xta_d[:, g0 * P:g0 * P + nsg_n])

                # DVE stream: masks + yt-scales interleaved (yt(g) emitted
                # after mask(g+2)); previous sg epilogue after.
                m_ts = {}

                def emit_mask(g):
                    # one is_equal covering groups g and g+1 (pair-batched:
                    # halves the per-op DVE fixed cost without the long
                    # serialization of a full-supergroup slab)
                    ge = min(g + 2, g1)
                    gilo = grp_insts[g][0]
                    gihi = grp_insts[ge - 1][1]
                    gni = gihi - gilo
                    m_t = mpool.tile([P, gni, P], F16,
                                     tag=f"m{(g // 2) % 3}")
                    sa = se_t[:, gilo * 2:gihi * 2]
                    in0 = bass.AP(sa.tensor, sa.offset,
                                  [sa.ap[0], [2, gni], [0, D], [1, 2]])
                    ia = io_t[:, :]
                    in1 = bass.AP(ia.tensor, ia.offset,
                                  [ia.ap[0], [0, gni], [2, D], [1, 2]])
                    ma = m_t[:, :, :]
                    mo = bass.AP(ma.tensor, ma.offset,
                                 [ma.ap[0], [P, gni], [2, D], [1, 2]])
                    nc.vector.tensor_tensor(out=mo, in0=in0, in1=in1,
                                            op=mybir.AluOpType.is_equal)
                    for gx in range(g, ge):
                        m_ts[gx] = (m_t, gilo)

                po = pso_pool.tile([D, SG_GROUPS, P], F32, tag="o",
                                   space="PSUM")
                yts = {}

                def emit_group_pe(g):
                    # instance matmuls: Yt_g = sum xgs_t^T @ mask_inst
                    gg = g - g0
                    m_t, mbase = m_ts[g]
                    py = psy_pool.tile([D, P], F32, tag=f"y{g % 3}",
                                       space="PSUM")
                    insts = ginsts[g]
                    kb = len(insts)
                    for j, (inst_id, t) in enumerate(insts):
                        nc.tensor.matmul(
                            out=py[:, :],
                            lhsT=xg_t[:, t, :],
                            rhs=m_t[:, inst_id - mbase, :],
                            start=(j == 0),
                            stop=(j == kb - 1),
                        )
                    return py

                def emit_group_yt(g, py):
                    # yts = Yt * dinvT (dest-side dinv), fp16
                    gg = g - g0
                    n0 = g * P
                    np_ = min(n0 + P, NSH) - n0
                    y_t = ypool.tile([D, P], F16, tag=f"t{g % 3}")
                    nc.vector.tensor_tensor(
                        out=y_t[:, :np_], in0=py[:, :np_],
                        in1=dvt_t[:, n0:n0 + np_],
                        op=mybir.AluOpType.mult)
                    yts[g] = y_t

                def emit_group_w1(g):
                    # outT_g = W1^T @ yts + W0a^T @ xta  (+bias via ones row)
                    gg = g - g0
                    n0 = g * P
                    np_ = min(n0 + P, NSH) - n0
                    nc.tensor.matmul(out=po[:, gg, :np_], lhsT=w1_t[:],
                                     rhs=yts[g][:, :np_],
                                     start=True, stop=False)
                    nc.tensor.matmul(out=po[:, gg, :np_], lhsT=w0a_t[:],
                                     rhs=xta_t[:, gg * P:gg * P + np_],
                                     start=False, stop=True)

                pys = {}
                for g in range(g0, g1):
                    if (g - g0) % 2 == 0:
                        emit_mask(g)
                    if g - 2 >= g0:
                        gp = g - 2
                        pys[gp] = emit_group_pe(gp)
                        emit_group_yt(gp, pys[gp])
                        emit_group_w1(gp)
                flush_pending()
                for g in range(max(g0, g1 - 2), g1):
                    pys[g] = emit_group_pe(g)
                    emit_group_yt(g, pys[g])
                    emit_group_w1(g)
                pending = (po, g0, g1)
            flush_pending()
    nc.compile()
    return nc


# ------------------------------------------------------------- host prep
def _prep_l1(row, w):
    """Per-core padded weight tables. Returns (kd, list of [P, NG*kd] f16)."""
    core = row // NSH
    data = []
    kd = 4
    for c in range(N_CORES):
        sel = core == c
        r_loc = (row[sel] - c * NSH).astype(np.int64)
        w_c = w[sel]
        counts = np.bincount(r_loc, minlength=NSH)
        kd = max(kd, int(counts.max()))
        data.append((r_loc, w_c, counts))
    kd = ((kd + 3) // 4) * 4
    out = []
    for r_loc, w_c, counts in data:
        offs = np.cumsum(counts) - counts
        order = np.argsort(r_loc, kind="stable")
        r_s = r_loc[order]
        w_s = w_c[order]
        k = np.arange(len(r_s)) - offs[r_s]
        wpad = np.zeros((NG * P, kd), np.float16)
        wpad[r_s, k] = w_s
        wbig = wpad.reshape(NG, P, kd).transpose(1, 0, 2).reshape(P, NG * kd)
        out.append(np.ascontiguousarray(wbig))
    return kd, out


def _prep_l2(row, col, w):
    """Core-equalized, group-padded L2 schedule (no buckets, no straddles:
    each group's slots are padded to full 128-tiles, so instance == tile).

    Returns (sched, percore) where percore[c] holds slot assignments.
    """
    core = col // NSH
    percore = []
    counts = np.zeros((N_CORES, NG), np.int64)
    for c in range(N_CORES):
        sel = core == c
        loc = col[sel] - c * NSH
        percore.append((loc // P, loc % P, row[sel], w[sel]))
        counts[c] = np.bincount(loc // P, minlength=NG)
    smax = counts.max(axis=0)            # [NG] equalized group sizes
    gtiles = -(-smax // P)               # tiles per group

    seg_slot = np.zeros(NG, np.int64)    # global slot of group start
    sg_tiles = []
    tile_base = []
    ginsts = []
    grp_insts = []
    sg_insts = []
    gslot = 0
    inst_id = 0
    for sg in range(NSG):
        g0, g1 = sg * SG_GROUPS, min((sg + 1) * SG_GROUPS, NG)
        tile_base.append(gslot // P)
        sg_slot0 = gslot
        lo_sg = inst_id
        for g in range(g0, g1):
            seg_slot[g] = gslot
            nt = int(gtiles[g])
            t0 = (gslot - sg_slot0) // P
            lo = inst_id
            inst_id += nt
            grp_insts.append((lo, inst_id))
            ginsts.append(tuple((lo + j, t0 + j) for j in range(nt)))
            gslot += nt * P
        sg_tiles.append((gslot - sg_slot0) // P)
        sg_insts.append((lo_sg, inst_id))
    tot_slots = gslot
    tot_tiles = tot_slots // P
    tot_inst = inst_id

    sched = (None, tuple(ginsts), tuple(sg_tiles), tuple(tile_base),
             tuple(sg_insts), tuple(grp_insts), 0, tot_inst, tot_tiles)

    # --- per-core slot assignment ---
    slots = []
    for c in range(N_CORES):
        g_e, d_e, r_e, w_e = percore[c]
        cnt = counts[c]
        offs = np.cumsum(cnt) - cnt
        order = np.argsort(g_e, kind="stable")
        inv = np.empty_like(order)
        inv[order] = np.arange(len(order))
        pos_in_seg = inv - offs[g_e]
        slot_e = seg_slot[g_e] + pos_in_seg

        k_e = slot_e % P
        gtile_e = slot_e // P
        # instance id == global tile id by construction
        inst_e = gtile_e

        slotenc = np.full((P, tot_inst), SENTINEL, np.float16)
        slotenc[k_e, inst_e] = d_e.astype(np.float16)
        slots.append({"slot": slot_e, "rows": r_e, "w": w_e,
                      "slotenc": np.repeat(slotenc, 2, axis=1)})
    return sched, slots


# ------------------------------------------------------------------ kernel()
def kernel(x, edge_index, edge_weight, W0, W1, b):
    global LAST_STATS
    x = np.asarray(x, np.float32)
    edge_index = np.asarray(edge_index)
    w = np.asarray(edge_weight, np.float32)
    W0 = np.asarray(W0, np.float32)
    W1 = np.asarray(W1, np.float32)
    b = np.asarray(b, np.float32)
    row = edge_index[0].astype(np.int64)
    col = edge_index[1].astype(np.int64)

    kd, wpads = _prep_l1(row, w)
    sched, slots = _prep_l2(row, col, w)
    tot_tiles = sched[8]
    sched_key = (sched[0], sched[2], sched[4], sched[6], sched[7], sched[8])

    if ("l1", kd) not in _cache:
        _cache[("l1", kd)] = build_l1(kd)
    nc1 = _cache[("l1", kd)]
    if ("l2", sched_key) not in _cache:
        _cache[("l2", sched_key)] = build_l2(sched)
    nc2 = _cache[("l2", sched_key)]

    in1 = [{"wpad": wpads[c]} for c in range(N_CORES)]
    res1 = run_bass_kernel_spmd(nc1, in1, core_ids=list(range(N_CORES)))

    # dinv per node (host assembly of the L1 output)
    dinv_full = np.empty(N_NODES, np.float32)
    for c in range(N_CORES):
        dv = res1.results[c]["dinv"]          # [P, NG], node = g*128 + p
        dinv_full[c * NSH:(c + 1) * NSH] = dv.T.reshape(-1)[:NSH]

    w1h = W1.astype(np.float16)
    w0a = np.concatenate([W0, b.reshape(1, D)], axis=0).astype(np.float16)
    iota = np.tile(np.arange(P, dtype=np.float16), (P, 1))

    in2 = []
    for c in range(N_CORES):
        s = slots[c]
        coef = (-s["w"] * dinv_full[s["rows"]]).astype(np.float32)
        xg = np.zeros((tot_tiles * P, D), np.float16)
        xg[s["slot"]] = (x[s["rows"]] * coef[:, None]).astype(np.float16)
        xgs = np.ascontiguousarray(
            xg.reshape(tot_tiles, P, D).transpose(1, 0, 2)).reshape(P, -1)

        xs = x[c * NSH:(c + 1) * NSH]
        xta = np.ascontiguousarray(np.concatenate(
            [xs.T, np.ones((1, NSH), np.float32)], axis=0).astype(np.float16))
        dvt = np.ascontiguousarray(np.tile(
            dinv_full[c * NSH:(c + 1) * NSH].astype(np.float16), (D, 1)))
        in2.append({"xgs": xgs, "xta": xta, "w0a": w0a, "w1": w1h,
                    "dinvT": dvt, "slotenc": s["slotenc"], "iota": iota})
    res2 = run_bass_kernel_spmd(nc2, in2, core_ids=list(range(N_CORES)))
    out = np.concatenate(
        [res2.results[c]["outT"].T for c in range(N_CORES)], axis=0)
    LAST_STATS = {
        "l1_exec_ns": res1.exec_time_ns,
        "l2_exec_ns": res2.exec_time_ns,
        "insts": sched[7],
        "tiles": sched[8],
    }
    return np.ascontiguousarray(out.astype(np.float32))
